# revision 56
# baseline (speedup 1.0000x reference)
"""Multi-head attention (B=2, N=2048, C=768, H=12) on 8 trn2 cores.

Sharding: core i handles batch b = i//4 and head-group g = i%4 (3 heads).
All device data is fp16 (tolerance 2e-2 allows it); matmul accumulation
stays fp32 in PSUM.

Per-core pipeline:
  1. QKV^T projection from host-pre-transposed xT [C, N]:
       q01/k01  [128, N]: heads 0,1 d-major (h0 at partitions 0:64, h1 at
                64:128) -> natural row-tile pairing for the score matmuls.
       q2d/k2d  [128, N]: head 2 duplicated in both partition halves so its
                score matmuls can be row-tile paired across adjacent k-chunks.
       v        [N, 65] per (k-chunk, head): cols 0:64 = v, col 64 = ones
                (softmax denominator trick).
  2. Scores transposed: S^T[k, q] = k_h^T-chunk.T @ q_h. Heads 0/1 (and for
     head 2, adjacent k-chunks) issue as K=64 matmuls at tile_position
     (0,0)/(64,0) -> they stream concurrently in the PE array.
  3. exp via ScalarE (the kernel's throughput floor: ~96 activations of
     [128,1024]); output fp16 to SBUF.
  4. attn@V with lhsT = [v | 1]: psum partial rows 0:64 = unnormalized
     attn_out^T, row 64 = denominators.  Partials cover GK=4 k-chunks in
     a 1-bank psum tile and are flushed (DVE copy/add) into per-head SBUF
     fp32 accumulators — this frees psum so the score ring gets THREE
     slots, which is what lets the row-tiled score matmul pairs actually
     be ready together and pack in the PE array (with 2 slots the serial
     exp chain skews their readiness by ~1us and they serialize).
     Normalize: gpsimd partition-broadcast of the denominator row, DVE
     reciprocal_approx_fast, DVE multiply -> fp16.
  5. Output exchange via per-window AllGathers of the NORMALIZED
     attention output (4x less data than ReduceScattering projection
     partials).  Each window w sends two chunks: h2 [64, QW] (ready a
     full phase before h01) and h01 [128, QW].  w_proj rows are
     host-permuted to the gathered order ([4 groups' h2 | 4 groups'
     h01-pairs]) so each core computes its c_out slice of the projection
     locally at full K=128 efficiency, bias folded into the psum->sbuf
     copy (DVE tensor_scalar add / ACT Identity).  AG(w0-h2)/AG(w0-h01)
     hide under phases B/C, AG(w1-h2) under phase D; only AG(w1-h01)
     (~16us) plus a short local projection is tail-exposed.

Scheduling: attention is one software pipeline in 4 phases (w0-h2,
w0-h01, w1-h2, w1-h01).  QKV projection groups and v-projection chunks
interleave into phases A/B as PE gap fillers; the w0 projection fills
phase D.  attn@V jobs trail the score/exp stream by one group so the
in-order PE queue never blocks on a partial slot waiting for a DVE
flush.  A tiny warmup collective at kernel start absorbs the ~60us
first-collective CC-core boot latency.  Input DMAs are consolidated
(one instruction per tensor / xT half) and split across the sync and
scalar HWDGE queues; the scalar queue only carries transfers that
finish before phase A's first exp needs it.
"""

import numpy as np

B, N, C, H, HD = 2, 2048, 768, 12, 64
G = 4              # tensor-parallel head groups
HL = H // G        # 3 heads per core
CHL = HL * HD      # 192 local channels
SCALE = HD ** -0.5
NCORES = 8
CT = C // 128      # 6 contraction chunks
FW = 512           # matmul free width (psum bank)
QW = 1024          # q window width
NWIN = N // QW     # 2 windows
KT = N // 128      # 16 k chunks
VW = HD + 1        # v tile cols: 64 v + 1 ones

_CACHE = {}


def _build_nc():
    import concourse.bass as bass
    import concourse.bacc as bacc
    import concourse.tile as tile
    import concourse.mybir as mybir

    F32 = mybir.dt.float32
    F16 = mybir.dt.float16
    AF = mybir.ActivationFunctionType
    RG = [[0, 1, 2, 3], [4, 5, 6, 7]]

    # All inputs host-preshuffled to partition-major layouts so every input
    # DMA is a contiguous full-line copy (the on-the-fly "(c p) m -> p c m"
    # rearrange produced 512B descriptor lines that ran at ~1/4 bandwidth
    # and stalled phase A behind the weight loads).
    nc = bacc.Bacc(num_devices=NCORES)
    NF = N // FW   # 4 f-quarters of xT
    xT_d = nc.declare_dram_parameter("xT", [128, NF * CT * FW], F16,
                                     isOutput=False)
    wqa_d = nc.declare_dram_parameter("wqa", [128, CT * 256], F16,
                                      isOutput=False)
    wka_d = nc.declare_dram_parameter("wka", [128, CT * 256], F16,
                                      isOutput=False)
    wv_d = nc.declare_dram_parameter("wv", [128, CT * CHL], F16,
                                     isOutput=False)
    wp_d = nc.declare_dram_parameter("wp", [128, CT * CHL], F16,
                                     isOutput=False)
    bp_d = nc.declare_dram_parameter("bp", [128, 2], F32, isOutput=False)
    out_d = nc.declare_dram_parameter("out", [CHL, N], F16, isOutput=True)

    with tile.TileContext(nc) as tc:
        with tc.tile_pool(name="dram", bufs=1, space="DRAM") as dram:
            # per window: h2 chunk [64, QW] and h01 chunk [128, QW]
            # (each collective has ~5-8us fixed cost, so heads 0/1 share)
            ag2_ins = [dram.tile([64, QW], F16, name=f"ag2_in{w}")
                       for w in range(NWIN)]
            ag2_outs = [dram.tile([256, QW], F16, name=f"ag2_out{w}")
                        for w in range(NWIN)]
            ag01_ins = [dram.tile([128, QW], F16, name=f"ag01_in{w}")
                        for w in range(NWIN)]
            ag01_outs = [dram.tile([512, QW], F16, name=f"ag01_out{w}")
                         for w in range(NWIN)]
            warm_in = dram.tile([8, 128], F16, name="warm_in")
            warm_out = dram.tile([32, 128], F16, name="warm_out")

            with tc.tile_pool(name="sb", bufs=1) as P, \
                    tc.tile_pool(name="ps", bufs=1, space="PSUM") as PS:
                # ---- input DMAs: consolidated (one instruction per
                # tensor / xT half), weights on the scalar HWDGE queue so
                # they stream in parallel with xT on the sync queue.  The
                # prologue needs only wka/wqa + xT cols 0:1024. ----
                wka_sb = P.tile([128, CT, 256], F16)
                wqa_sb = P.tile([128, CT, 256], F16)
                # xT as [f-quarter, ct, fw]
                xT_sb = P.tile([128, NF, CT, FW], F16)
                nc.scalar.dma_start(out=wka_sb[:], in_=wka_d[:, :])
                nc.scalar.dma_start(out=wqa_sb[:], in_=wqa_d[:, :])
                wv_sb = P.tile([128, CT, CHL], F16)
                nc.scalar.dma_start(out=wv_sb[:], in_=wv_d[:, :])
                FQ = CT * FW
                for f in range(NF):
                    nc.sync.dma_start(out=xT_sb[:, f, :, :],
                                      in_=xT_d[:, f * FQ:(f + 1) * FQ])
                # w_proj column-slice [C, 192], rows host-permuted to the
                # gathered order.  On the sync queue (not scalar): a long
                # transfer on the scalar HWDGE queue would block phase A's
                # exps behind it (strict FIFO).
                wp_sb = P.tile([128, CT, CHL], F16)
                nc.sync.dma_start(out=wp_sb[:], in_=wp_d[:, :])
                bp_sb = P.tile([128, 2], F32)
                nc.sync.dma_start(out=bp_sb[:], in_=bp_d[:, :])
                warm_sb = P.tile([8, 128], F16)
                ones64 = P.tile([1, HD], F16)
                nc.vector.memset(ones64[:], 1.0)
                ones128 = P.tile([128, 1], F16)
                nc.vector.memset(ones128[:], 1.0)

                # ---- persistent QKV results ----
                k01_sb = P.tile([128, N], F16)
                q01_sb = P.tile([128, N], F16)
                k2d_sb = P.tile([128, N], F16)
                q2d_sb = P.tile([128, N], F16)
                # [n, kt, h, VW]: cols 0:64 = v, col 64 = ones
                v_sb = P.tile([128, KT, HL, VW], F16)
                nc.vector.memset(v_sb[:, :, :, HD:VW], 1.0)
                # gathered attention output, [c_in chunk, kc, q]
                atf_sb = [P.tile([128, CT, QW], F16, name=f"atf{w}")
                          for w in range(NWIN)]

                # ---- QKV projection emitters (interleaved into attention
                # as PE gap-fillers: keeps TensorE dense so HAM stays at
                # full clock) ----
                # psum ring "sc": [128,1024] slots (2 banks) x3 -> qk/v
                # projection psum, score tiles, and proj psum.  THREE slots
                # give the two row-tiled score matmuls of an iteration
                # enough runway to be ready simultaneously (the serial exp
                # chain frees slots ~1us apart), so they actually pair in
                # the PE array instead of serializing.
                # psum ring "ac": [128,512] slots (1 bank) x2 -> short-lived
                # attn@V partials, flushed into SBUF fp32 accumulators.
                def qk_group(dst, wsb, mlo, f):
                    qk_ps = PS.tile([128, FW], F32, tag="sc", bufs=3,
                                    padded_shape=[128, QW], name="qk_ps")
                    for ct in range(CT):
                        nc.tensor.matmul(
                            qk_ps[:],
                            lhsT=wsb[:, ct, mlo:mlo + 128],
                            rhs=xT_sb[:, f, ct, :],
                            start=(ct == 0), stop=(ct == CT - 1),
                        )
                    nc.vector.tensor_copy(
                        dst[:, f * FW:(f + 1) * FW], qk_ps[:])

                def v_group(nt):
                    v_ps = PS.tile([128, CHL], F32, tag="sc", bufs=3,
                                   padded_shape=[128, QW], name="v_ps")
                    nf, nr = divmod(nt * 128, FW)
                    for ct in range(CT):
                        nc.tensor.matmul(
                            v_ps[:],
                            lhsT=xT_sb[:, nf, ct, nr:nr + 128],
                            rhs=wv_sb[:, ct, :],
                            start=(ct == 0), stop=(ct == CT - 1),
                        )
                    nc.vector.tensor_copy(
                        v_sb[:, nt, :, 0:HD],
                        v_ps[:].rearrange("p (h d) -> p h d", h=HL))

                # tiny warmup collective: absorbs the ~60us first-collective
                # CC-core boot latency long before the real AllGathers.
                nc.vector.memset(warm_sb[:], 0.0)
                nc.sync.dma_start(out=warm_in[:], in_=warm_sb[:])
                nc.gpsimd.collective_compute(
                    "AllGather",
                    mybir.AluOpType.bypass,
                    replica_groups=RG,
                    ins=[warm_in.opt()],
                    outs=[warm_out.opt()],
                )

                # prologue: only what the first score pair needs (k01 f0
                # covers k-chunks 0-3; q01 f0/f1 cover window 0); the rest
                # ride as phase-A/B fillers, 2 iterations ahead of use
                qk_group(k01_sb, wka_sb, 0, 0)
                qk_group(q01_sb, wqa_sb, 0, 0)
                qk_group(q01_sb, wqa_sb, 0, 1)

                # ---- attention + AllGather + local projection ----
                with tc.tile_pool(name="att_sb", bufs=1) as AS:
                    def scores_pair(w, lhs_tile, rhs_tile, kc0, kc1, Sa, Sb):
                        """Two K=64 score matmuls row-tiled (0,0)/(64,0)."""
                        q0 = w * QW
                        for j in range(QW // FW):
                            js = slice(q0 + j * FW, q0 + (j + 1) * FW)
                            ps_js = slice(j * FW, (j + 1) * FW)
                            nc.tensor.matmul(
                                Sa[:, ps_js],
                                lhsT=lhs_tile[0:64, kc0 * 128:(kc0 + 1) * 128],
                                rhs=rhs_tile[0:64, js],
                            )
                            nc.tensor.matmul(
                                Sb[:, ps_js],
                                lhsT=lhs_tile[64:128, kc1 * 128:(kc1 + 1) * 128],
                                rhs=rhs_tile[64:128, js],
                            )

                    def run_job(job):
                        """Accumulate one group of kcs for one head into a
                        1-bank psum partial per j, then flush (copy/add)
                        into the head's SBUF fp32 accumulator on DVE."""
                        h, Af, g, Es_ = job
                        for j in range(QW // FW):
                            js = slice(j * FW, (j + 1) * FW)
                            p = PS.tile([VW, FW], F32, tag="ac", bufs=2,
                                        padded_shape=[128, FW],
                                        name=f"p{h}g{g}")
                            for i, (E, kc) in enumerate(Es_):
                                nc.tensor.matmul(
                                    p[:],
                                    lhsT=v_sb[:, kc, h, :],
                                    rhs=E[:, js],
                                    start=(i == 0), stop=(i == len(Es_) - 1),
                                )
                            if g == 0:
                                nc.vector.tensor_copy(Af[:, js], p[:])
                            else:
                                nc.vector.tensor_add(Af[:, js], Af[:, js],
                                                     p[:])

                    def run_job01(job):
                        """Joint h0/h1 attn@V group: the two heads' AV
                        matmuls col-pack (64|64) into ONE stream slot, and
                        their denominators ride a second slot as four
                        concurrent col-tiled 1-column matmuls (even kcs on
                        rows 0/32, odd kcs on rows 64/96; the flush sums
                        the split rows).  3 stream slots per 2 k-chunks
                        instead of 4 with the per-head ones-column trick."""
                        A0f, A1f, g, Es_ = job
                        n = len(Es_)
                        for j in range(QW // FW):
                            js = slice(j * FW, (j + 1) * FW)
                            p = PS.tile([128, FW], F32, tag="ac", bufs=2,
                                        padded_shape=[128, FW], name=f"pj{g}")
                            pd = PS.tile([128, FW], F32, tag="ac", bufs=2,
                                         padded_shape=[128, FW],
                                         name=f"pd{g}")
                            for i, (E0, E1, kc) in enumerate(Es_):
                                st, sp = i == 0, i == n - 1
                                nc.tensor.matmul(
                                    p[0:64, :], lhsT=v_sb[:, kc, 0, 0:HD],
                                    rhs=E0[:, js], start=st, stop=sp,
                                    tile_position=(0, 0))
                                nc.tensor.matmul(
                                    p[64:128, :], lhsT=v_sb[:, kc, 1, 0:HD],
                                    rhs=E1[:, js], start=st, stop=sp,
                                    tile_position=(0, 64))
                                ro = 64 * (i % 2)
                                st2 = i < 2
                                sp2 = i >= n - 2
                                nc.tensor.matmul(
                                    pd[ro:ro + 1, :], lhsT=ones128[:],
                                    rhs=E0[:, js], start=st2, stop=sp2,
                                    tile_position=(0, ro))
                                nc.tensor.matmul(
                                    pd[ro + 32:ro + 33, :], lhsT=ones128[:],
                                    rhs=E1[:, js], start=st2, stop=sp2,
                                    tile_position=(0, ro + 32))
                            for Af, lo in ((A0f, 0), (A1f, 64)):
                                if g == 0:
                                    nc.vector.tensor_copy(
                                        Af[0:64, js], p[lo:lo + 64, :])
                                else:
                                    nc.vector.tensor_add(
                                        Af[0:64, js], Af[0:64, js],
                                        p[lo:lo + 64, :])
                            for Af, d in ((A0f, 0), (A1f, 32)):
                                if g == 0:
                                    nc.vector.tensor_copy(
                                        Af[64:65, js], pd[d:d + 1, :])
                                else:
                                    nc.vector.tensor_add(
                                        Af[64:65, js], Af[64:65, js],
                                        pd[d:d + 1, :])
                                nc.vector.tensor_add(
                                    Af[64:65, js], Af[64:65, js],
                                    pd[d + 64:d + 65, :])

                    def normalize(Af, w, h):
                        """ag in rows for head h = Af[0:64] / Af[64]."""
                        at = AS.tile([64, QW], F16, tag="at", bufs=3)
                        for j in range(QW // FW):
                            js = slice(j * FW, (j + 1) * FW)
                            den = AS.tile([1, FW], F32, tag="den", bufs=4)
                            bcs = AS.tile([64, FW], F32, tag="bcs", bufs=4)
                            rcp = AS.tile([64, FW], F32, tag="rcp", bufs=4)
                            nc.vector.tensor_copy(den[:], Af[64:65, js])
                            nc.gpsimd.partition_broadcast(bcs[:], den[:])
                            nc.vector.reciprocal_approx_fast(rcp[:], bcs[:])
                            nc.vector.tensor_mul(at[:, js], Af[0:64, js],
                                                 rcp[:])
                        if h == 2:
                            nc.sync.dma_start(out=ag2_ins[w][:, :], in_=at[:])
                        else:
                            nc.sync.dma_start(
                                out=ag01_ins[w][h * 64:(h + 1) * 64, :],
                                in_=at[:])

                    def all_gather(w, part):
                        ins = ag2_ins if part == 2 else ag01_ins
                        outs = ag2_outs if part == 2 else ag01_outs
                        nc.gpsimd.collective_compute(
                            "AllGather",
                            mybir.AluOpType.bypass,
                            replica_groups=RG,
                            ins=[ins[w].opt()],
                            outs=[outs[w].opt()],
                        )

                    def atf_dma(w, j, parts=(2, 0)):
                        """Fetch gathered at_full half-window into SBUF.
                        c_in chunks 0:2 <- h2 gather, 2:6 <- h01 gather
                        (w_proj rows are host-permuted to match)."""
                        js = slice(j * FW, (j + 1) * FW)
                        if 2 in parts:
                            nc.sync.dma_start(
                                out=atf_sb[w][:, 0:2, js],
                                in_=ag2_outs[w][:, js].rearrange(
                                    "(c p) n -> p c n", p=128))
                        if 0 in parts:
                            nc.sync.dma_start(
                                out=atf_sb[w][:, 2:CT, js],
                                in_=ag01_outs[w][:, js].rearrange(
                                    "(c p) n -> p c n", p=128))

                    def proj_mms(pr, w, m, msz, kcs, start, stop):
                        mlo = m * 128
                        for j in range(QW // FW):
                            ps_js = slice(j * FW, (j + 1) * FW)
                            for i, kc in enumerate(kcs):
                                nc.tensor.matmul(
                                    pr[:, ps_js],
                                    lhsT=wp_sb[:, kc, mlo:mlo + msz],
                                    rhs=atf_sb[w][:, kc, ps_js],
                                    start=(start and i == 0),
                                    stop=(stop and i == len(kcs) - 1),
                                )

                    def proj_fin(pr, w, m):
                        mlo = m * 128
                        msz = min(128, CHL - mlo)
                        po = AS.tile([msz, QW], F16, tag="po", bufs=3)
                        for j in range(QW // FW):
                            ps_js = slice(j * FW, (j + 1) * FW)
                            if w == 1 and m == 0:
                                # at the tail ACT is idle: it takes one
                                # m-chunk's bias-add while DVE takes the
                                # other, in parallel (mid-kernel ACT is
                                # exp-saturated, so w0 stays off scalar)
                                nc.scalar.activation(
                                    po[:, ps_js], pr[:, ps_js], AF.Identity,
                                    bias=bp_sb[0:msz, m:m + 1])
                            else:
                                nc.vector.tensor_scalar_add(
                                    po[:, ps_js], pr[:, ps_js],
                                    bp_sb[0:msz, m:m + 1])
                            nc.sync.dma_start(
                                out=out_d[mlo:mlo + msz,
                                          w * QW + j * FW:
                                          w * QW + (j + 1) * FW],
                                in_=po[:, ps_js])

                    def proj_m(w, m):
                        """out^T[m-chunk of local c-slice, window w]."""
                        msz = min(128, CHL - m * 128)
                        pr = PS.tile([msz, QW], F32, tag="sc", bufs=3,
                                     padded_shape=[128, QW], name="pr")
                        proj_mms(pr, w, m, msz, range(CT), True, True)
                        proj_fin(pr, w, m)

                    GK = 4  # k-chunks per attn@V accumulation group

                    def attn_h2(w, interleave):
                        """Head 2, adjacent-k-chunk row-tile-paired; attn@V
                        accumulates GK k-chunks per psum partial, flushed to
                        the SBUF accumulator (run_job).  Jobs trail the
                        score/exp stream so the in-order PE queue never
                        blocks on a partial slot waiting for a flush."""
                        A2f = AS.tile([VW, QW], F32, tag="Af", bufs=3,
                                      name=f"A2f_{w}")
                        Eq, jobs, g = [], [], [0]
                        def maybe_job():
                            if len(Eq) == GK:
                                jobs.append((2, A2f, g[0], list(Eq)))
                                Eq.clear()
                                g[0] += 1
                        for kcp in range(KT // 2):
                            kc0, kc1 = 2 * kcp, 2 * kcp + 1
                            Se = PS.tile([128, QW], F32, tag="sc", bufs=3)
                            So = PS.tile([128, QW], F32, tag="sc", bufs=3)
                            scores_pair(w, k2d_sb, q2d_sb, kc0, kc1, Se, So)
                            Ee = AS.tile([128, QW], F16, tag="E", bufs=18)
                            Eo = AS.tile([128, QW], F16, tag="E", bufs=18)
                            nc.scalar.activation(Ee[:], Se[:], AF.Exp,
                                                 scale=SCALE)
                            nc.scalar.activation(Eo[:], So[:], AF.Exp,
                                                 scale=SCALE)
                            Eq += [(Ee, kc0), (Eo, kc1)]
                            maybe_job()
                            interleave(kcp)
                            while jobs:
                                run_job(jobs.pop(0))
                        for job in jobs:
                            run_job(job)
                        return A2f

                    def attn_h01(w, interleave):
                        """Heads 0/1, head-row-tile-paired scores; joint
                        col-packed attn@V jobs (see run_job01)."""
                        A0f = AS.tile([VW, QW], F32, tag="Af", bufs=3,
                                      name=f"A0f_{w}")
                        A1f = AS.tile([VW, QW], F32, tag="Af", bufs=3,
                                      name=f"A1f_{w}")
                        Eq, jobs, g = [], [], [0]
                        for kc in range(KT):
                            S0 = PS.tile([128, QW], F32, tag="sc", bufs=3)
                            S1 = PS.tile([128, QW], F32, tag="sc", bufs=3)
                            scores_pair(w, k01_sb, q01_sb, kc, kc, S0, S1)
                            E0 = AS.tile([128, QW], F16, tag="E", bufs=18)
                            E1 = AS.tile([128, QW], F16, tag="E", bufs=18)
                            nc.scalar.activation(E0[:], S0[:], AF.Exp,
                                                 scale=SCALE)
                            nc.scalar.activation(E1[:], S1[:], AF.Exp,
                                                 scale=SCALE)
                            Eq.append((E0, E1, kc))
                            if len(Eq) == GK:
                                for h, Af in ((0, A0f), (1, A1f)):
                                    jobs.append((h, Af, g[0],
                                                 [(es[h], kc_)
                                                  for *es, kc_ in Eq]))
                                g[0] += 1
                                Eq.clear()
                            interleave(kc)
                            while jobs:
                                run_job(jobs.pop(0))
                        for job in jobs:
                            run_job(job)
                        return A0f, A1f

                    def fast_normalize(heads, w):
                        """Latency-optimized normalize for a phase seam on
                        the critical path of an AllGather trigger: chains
                        are issued interleaved across heads and the
                        denominator is partition-broadcast on the PE
                        instead of gpsimd."""
                        dens, bcss, rcps = [], [], []
                        for i, (h, A) in enumerate(heads):
                            den = AS.tile([1, QW], F16, tag="dent", bufs=2,
                                          name=f"dent{h}")
                            # second head's copy on gpsimd (all-SBUF) so the
                            # two chains start in parallel
                            eng = nc.vector if i == 0 else nc.gpsimd
                            eng.tensor_copy(den[:], A[64:65, :])
                            dens.append(den)
                        for i, (h, A) in enumerate(heads):
                            bcs = PS.tile([64, QW], F32, tag="sc", bufs=3,
                                          padded_shape=[128, QW],
                                          name=f"bcst{h}")
                            for j in range(QW // FW):
                                js = slice(j * FW, (j + 1) * FW)
                                nc.tensor.matmul(bcs[:, js], lhsT=ones64[:],
                                                 rhs=dens[i][:, js])
                            bcss.append(bcs)
                        for i, (h, A) in enumerate(heads):
                            rcp = AS.tile([64, QW], F32, tag="rcpt", bufs=2,
                                          name=f"rcpt{h}")
                            for j in range(QW // FW):
                                js = slice(j * FW, (j + 1) * FW)
                                nc.vector.reciprocal_approx_fast(
                                    rcp[:, js], bcss[i][:, js])
                            rcps.append(rcp)
                        for i, (h, A) in enumerate(heads):
                            at = AS.tile([64, QW], F16, tag="at", bufs=3,
                                         name=f"att{h}")
                            eng = nc.vector if i == 0 else nc.gpsimd
                            for j in range(QW // FW):
                                js = slice(j * FW, (j + 1) * FW)
                                eng.tensor_mul(at[:, js], A[0:64, js],
                                               rcps[i][:, js])
                            if h == 2:
                                nc.sync.dma_start(out=ag2_ins[w][:, :],
                                                  in_=at[:])
                            else:
                                nc.sync.dma_start(
                                    out=ag01_ins[w][h * 64:(h + 1) * 64, :],
                                    in_=at[:])

                    # phase A: w0 heads 0/1; all v chunks + the other q/k
                    # projections as fillers, 2 iterations ahead of use.
                    def fillA(kc):
                        if kc < 8:
                            v_group(2 * kc)
                            v_group(2 * kc + 1)
                        if kc < 3:
                            # k01 f(kc+1): consumed by score pairs at
                            # iteration 4*(kc+1)
                            qk_group(k01_sb, wka_sb, 0, kc + 1)
                        elif kc == 3:
                            qk_group(k2d_sb, wka_sb, 128, 0)
                        elif kc == 4:
                            qk_group(q2d_sb, wqa_sb, 128, 0)
                        elif kc == 5:
                            qk_group(q2d_sb, wqa_sb, 128, 1)
                    A0, A1 = attn_h01(0, fillA)
                    # phase B: w0 head 2; rest of the q/k projections.
                    # AG(w0-h01) fires at phase B start and hides under B
                    # (the CC cores boot ~66us in, absorbed by the warmup).
                    normalize(A0, 0, 0)
                    normalize(A1, 0, 1)
                    all_gather(0, 0)
                    fillB = [
                        lambda: qk_group(k2d_sb, wka_sb, 128, 1),
                        lambda: qk_group(k2d_sb, wka_sb, 128, 2),
                        lambda: qk_group(k2d_sb, wka_sb, 128, 3),
                        lambda: qk_group(q01_sb, wqa_sb, 0, 2),
                        lambda: qk_group(q01_sb, wqa_sb, 0, 3),
                        lambda: qk_group(q2d_sb, wqa_sb, 128, 2),
                        lambda: qk_group(q2d_sb, wqa_sb, 128, 3),
                    ]
                    A2 = attn_h2(
                        0, lambda kcp: fillB[kcp]()
                        if kcp < len(fillB) else None)
                    # phase C: w1 heads 0/1; AG(w0-h2) fires here
                    normalize(A2, 0, 2)
                    all_gather(0, 2)
                    A0, A1 = attn_h01(1, lambda kc: None)
                    # seam: fast normalize of w1 h0/h1 so AG(w1-h01) can
                    # hide under phase D
                    fast_normalize([(0, A0), (1, A1)], 1)
                    all_gather(1, 0)

                    # phase D: w1 head 2 (lean: the w0 projection moved to
                    # the tail AG-wait window, off the HAM-throttled phase)
                    def fillD(kcp):
                        if kcp == 0:
                            for j in range(QW // FW):
                                atf_dma(0, j)
                    A2 = attn_h2(1, fillD)

                    # ---- tail: normalize w1 h2, small AllGather, proj ----
                    # The PE fills the final AllGather's flight time with
                    # the whole w0 projection plus the h01 chunks of the w1
                    # projection; only the 2 h2 k-chunks + bias remain
                    # after it lands.
                    fast_normalize([(2, A2)], 1)
                    all_gather(1, 2)
                    proj_m(0, 0)
                    proj_m(0, 1)
                    for j in range(QW // FW):
                        atf_dma(1, j, parts=(0,))
                    pr0 = PS.tile([128, QW], F32, tag="sc", bufs=3,
                                  padded_shape=[128, QW], name="pr0")
                    pr1 = PS.tile([64, QW], F32, tag="sc", bufs=3,
                                  padded_shape=[128, QW], name="pr1")
                    proj_mms(pr0, 1, 0, 128, range(2, CT), True, False)
                    proj_mms(pr1, 1, 1, 64, range(2, CT), True, False)
                    for j in range(QW // FW):
                        atf_dma(1, j, parts=(2,))
                    proj_mms(pr0, 1, 0, 128, range(0, 2), False, True)
                    proj_mms(pr1, 1, 1, 64, range(0, 2), False, True)
                    proj_fin(pr0, 1, 0)
                    proj_fin(pr1, 1, 1)
    nc.finalize()
    return nc


def get_nc():
    if "nc" not in _CACHE:
        _CACHE["nc"] = _build_nc()
    return _CACHE["nc"]


def _pre(w):
    """[C, M] -> partition-major [128, CT*M] so the device DMA is one
    contiguous full-line copy."""
    m = w.shape[1]
    return np.ascontiguousarray(
        w.reshape(CT, 128, m).transpose(1, 0, 2).reshape(128, CT * m),
        dtype=np.float16)


def make_in_maps(x, w_qkv, w_proj, b_proj):
    x = np.asarray(x, dtype=np.float32)
    w_qkv = np.asarray(w_qkv, dtype=np.float32)
    w_proj = np.asarray(w_proj, dtype=np.float32)
    b_proj = np.asarray(b_proj, dtype=np.float32)
    in_maps = []
    for core in range(NCORES):
        b, g = divmod(core, G)
        cs = slice(g * CHL, (g + 1) * CHL)
        wq = w_qkv[:, 0 * C:1 * C][:, cs]
        wk = w_qkv[:, 1 * C:2 * C][:, cs]
        wv = w_qkv[:, 2 * C:3 * C][:, cs]
        # [heads01 | head2 | head2-dup]
        wqa = np.concatenate([wq[:, 0:128], wq[:, 128:192], wq[:, 128:192]],
                             axis=1)
        wka = np.concatenate([wk[:, 0:128], wk[:, 128:192], wk[:, 128:192]],
                             axis=1)
        # bias for the local c_out slice, [128, 2] column-per-m-chunk
        bp = np.zeros((128, 2), dtype=np.float32)
        bp[:, 0] = b_proj[cs][0:128]
        bp[0:64, 1] = b_proj[cs][128:192]
        # w_proj rows permuted to the gathered at_full order:
        # [4 groups' h2 | 4 groups' (h0, h1)]
        head_order = [2, 5, 8, 11, 0, 1, 3, 4, 6, 7, 9, 10]
        row_perm = np.concatenate(
            [np.arange(h * HD, (h + 1) * HD) for h in head_order])
        # xT [C, N] -> [128, f-quarter, ct, fw] flattened
        xT = x[b].T.reshape(CT, 128, N // FW, FW)
        xT = xT.transpose(1, 2, 0, 3).reshape(128, -1)
        im = {
            "xT": np.ascontiguousarray(xT, dtype=np.float16),
            "wqa": _pre(wqa),
            "wka": _pre(wka),
            "wv": _pre(wv),
            "wp": _pre(w_proj[row_perm][:, cs]),
            "bp": bp,
        }
        in_maps.append(im)
    return in_maps


def unshard(results):
    out = np.empty((B, N, C), dtype=np.float32)
    for b in range(B):
        outT = np.concatenate(
            [np.asarray(results[b * G + g]["out"], dtype=np.float32)
             for g in range(G)], axis=0)
        out[b] = outT.T
    return out


def kernel(x, w_qkv, w_proj, b_proj):
    from concourse.bass_utils import run_bass_kernel_spmd

    nc = get_nc()
    in_maps = make_in_maps(x, w_qkv, w_proj, b_proj)
    res = run_bass_kernel_spmd(nc, in_maps, list(range(NCORES)))
    return unshard(res.results)
